# revision 66
# baseline (speedup 1.0000x reference)
"""Trainium2 Bass kernel for nn_Convolution_1176821039249.

Computes out = base_map * mean_k box_k(x) for k in {3,5,7,9,11,13,15} with
replicate padding, on 8 NeuronCores in a 4x2 spatial grid (1024x2048 per
core plus a 7-pixel halo on all sides, sliced host-side from the padded
full image, so no device-side halo exchange is needed).

Algorithm (per core):
  The total 2D kernel K(di,dj) = sum_k 1/(7k^2) 1[|di|<=k//2] 1[|dj|<=k//2]
  is decomposed over the horizontal "wing" basis
      T_0 = x(center),  T_m(j) = x(j-m) + x(j+m)   (m = 1..7)
  so that  out = sum_{b=0..7} P_b-vertical-band applied to T_b, where
      P_b(d) = sum_{k: k//2 >= max(b,|d|)} 1/(7k^2).
  Wings m=1..6 are fused DVE tensor_tensor adds (3D fan views with
  column-offset steps -1/+1); wing 7 is column-split between DVE and Pool
  to balance engine load.  The vertical pyramid bands are 8 PSUM-accumulated
  banded matmuls on the PE per 114-row tile (per-512-col-chunk PSUM tiles so
  chunk drains never block later matmuls); ACT drains PSUM to fp16, Pool
  multiplies by base_map, and the fp16 result is stored (host upconverts to
  fp32).

Scheduling: the pipeline is paced by the DVE wing chain (its per-tile work
matches the PE's), so mid tiles compute wings and consume them half-major
(two 1024-col halves) -- halving the producer->consumer latency granularity
removes all mid-stream PE stalls, which matters doubly because any PE idle
gap resets the p-state ramp (1 low + ~6 mid-rate matmuls per gap).  Tile 0
quarter-splits wings (matching the small leading slice of the first x load
so the DVE chain starts ASAP), tile 1 half-splits them, and the last tile
quarter-splits so its per-chunk epilogues pipeline.  A long warm-up matmul
chain keeps the PE busy (and the p-state warm) through the fill.  The tail
batches stores (pair-chunk acc tiles), runs early-chunk multiplies on Pool,
and fuses the last two chunks' drain+multiply into DVE PSUM-read ops, since
each tail store costs ~650ns SP issue + ~625ns HWDGE serialization and each
ACT drain adds a sem round-trip right where the kernel is latency-bound.
"""

import numpy as np

F16 = np.float16

H = W = 4096
PAD = 7
GR, GC = 4, 2               # core grid: 4 row-groups x 2 col-groups
RPC = H // GR               # 1024 output rows per core
CPC = W // GC               # 2048 output cols per core
SHARD_R = RPC + 2 * PAD     # 1038
SHARD_C = CPC + 2 * PAD     # 2062
TILE_M = 114                # output rows per tile (128 - 2*PAD)
N_TILES = 9                 # 8 * 114 + 112 = 1024
LAST_M = RPC - (N_TILES - 1) * TILE_M   # 112
CHUNK = 512                 # matmul N chunk (one PSUM bank of fp32)
NCH = CPC // CHUNK          # 4 chunks per tile
N_CORES = 8
KERNEL_SIZES = (3, 5, 7, 9, 11, 13, 15)
DVE_W7 = 696                # columns of wing-7 computed on DVE; rest on Pool

_CACHE = {}


def _bands_np() -> np.ndarray:
    """lhsT band matrices, [128, 8*TILE_M] fp16.

    Band b column i row p holds P_b(p - i - 7): the vertical pyramid profile
    applied to wing tensor T_b.
    """
    w = {k: 1.0 / (7.0 * k * k) for k in KERNEL_SIZES}
    P = np.zeros((8, 15), dtype=np.float64)
    for b in range(8):
        for d in range(-7, 8):
            P[b, d + 7] = sum(w[k] for k in KERNEL_SIZES if k // 2 >= max(b, abs(d)))
    M = np.zeros((128, 8 * TILE_M), dtype=np.float64)
    for b in range(8):
        for i in range(TILE_M):
            for p in range(i, i + 15):
                M[p, b * TILE_M + i] = P[b, p - i]
    return M.astype(F16)


def _build_nc():
    import concourse.bass as bass
    import concourse.mybir as mybir
    import concourse.tile as tile

    dt = mybir.dt

    nc = bass.Bass()
    xb_d = nc.declare_dram_parameter("xb", [SHARD_R, SHARD_C], dt.float16, isOutput=False)
    base_d = nc.declare_dram_parameter("base", [RPC, CPC], dt.float16, isOutput=False)
    bands_d = nc.declare_dram_parameter("bands", [128, 8 * TILE_M], dt.float16, isOutput=False)
    out_d = nc.declare_dram_parameter("out", [RPC, CPC], dt.float16, isOutput=True)

    with tile.TileContext(nc) as tc:
        with (
            tc.tile_pool(name="const", bufs=1) as constp,
            tc.tile_pool(name="xin", bufs=3) as xpool,
            tc.tile_pool(name="wings", bufs=3) as wpool,
            tc.tile_pool(name="wq", bufs=4) as wqpool,
            tc.tile_pool(name="io", bufs=3) as iopool,
            tc.tile_pool(name="psum", bufs=2, space="PSUM") as psump,
        ):
            def ps_tile(c):
                return psump.tile([128, CHUNK], dt.float32, tag=f"ps{c}",
                                  name=f"ps{c}")

            # PE p-state warm-up: a serialized chain of small matmuls (RAW on
            # the same PSUM region) keeps the PE continuously busy through the
            # DMA fill phase so real matmuls arrive in a warm >3us stretch at
            # full 2.4 GHz instead of cold-burst p-states
            warm = constp.tile([128, 128], dt.float16, name="warm")
            nc.gpsimd.memset(warm[:], 0.0)
            warm_ps = ps_tile(0)
            for _ in range(52):
                nc.tensor.matmul(warm_ps[:16, :128], warm[:, :16], warm[:, :128],
                                 start=True, stop=True)

            bands_sb = constp.tile([128, 8 * TILE_M], dt.float16, name="bands_sb")

            def fan(src, start, n, step, width):
                # [K, n, width] view: slice i covers columns
                # start + i*step .. +width (overlapping windows)
                v = src[:, start:start + width].unsqueeze(1)
                lst = v.ap
                lst[1] = (step, n)
                v.ap = lst
                return v

            pending = None  # (ps_list, bt, r0, M) awaiting drain+mul+store

            def epilogue(tail=False):
                ps_list, bt, r0, M = pending
                acc = iopool.tile([128, CPC], dt.float16, tag="acc", name="acc")
                if tail:
                    # final deferred tile: per-chunk drain -> DVE mul -> store
                    # (DVE idles once wings are done; Pool is busy with the
                    # last tile's w7 quarters)
                    for c in range(NCH):
                        cs = slice(c * CHUNK, (c + 1) * CHUNK)
                        nc.scalar.copy(acc[:M, cs], ps_list[c][:M, :])
                        nc.vector.tensor_mul(acc[:M, cs], acc[:M, cs], bt[:M, cs])
                        if c % 2 == 1:
                            # half-batched stores: halves the SP issue +
                            # HWDGE serialization in the tail window
                            hs = slice((c - 1) * CHUNK, (c + 1) * CHUNK)
                            nc.sync.dma_start(out_d[r0:r0 + M, hs], acc[:M, hs])
                    return
                for c in range(NCH):
                    cs = slice(c * CHUNK, (c + 1) * CHUNK)
                    nc.scalar.copy(acc[:M, cs], ps_list[c][:M, :])
                nc.gpsimd.tensor_mul(acc[:M, :], acc[:M, :], bt[:M, :])
                nc.sync.dma_start(out_d[r0:r0 + M, :], acc[:M, :])

            HALF = CPC // 2
            for t in range(N_TILES):
                M = TILE_M if t < N_TILES - 1 else LAST_M
                K = M + 2 * PAD
                r0 = t * TILE_M
                last = t == N_TILES - 1
                ends = t <= 1 or last

                xt = xpool.tile([128, SHARD_C], dt.float16, tag="xt", name="xt")
                if t == 0:
                    # split the first load (one small slice covering the
                    # first wing quarter, then the rest): the DVE chain that
                    # paces the whole pipeline starts on x quarter 0
                    FIRST = CHUNK + 2 * PAD
                    nc.sync.dma_start(xt[:K, :FIRST], xb_d[r0:r0 + K, :FIRST])
                    nc.sync.dma_start(xt[:K, FIRST:], xb_d[r0:r0 + K, FIRST:])
                else:
                    nc.sync.dma_start(xt[:K, :], xb_d[r0:r0 + K, :])
                bt = iopool.tile([128, CPC], dt.float16, tag="bt", name="bt")
                nc.sync.dma_start(bt[:M, :], base_d[r0:r0 + M, :])
                if t == 0:
                    nc.sync.dma_start(bands_sb[:], bands_d[:])

                # wings m=1..6 fused: fan slice i reads cols (base-i ..) and
                # (base'+i ..), i.e. x shifted by -(i+1)/+(i+1) around PAD=7
                if ends:
                    # split wings in per-unit tiles: chunk c's matmuls depend
                    # only on the unit covering its columns, shortening
                    # fill/tail.  Tile 0 leads with a chunk-wide unit (it
                    # matches the small leading x slice) then widens; tiles
                    # 1 and 8 use halves -- fewer op launches on the
                    # chain-critical DVE.
                    bounds = [0, HALF, CPC] if t == 1 \
                        else [0, CHUNK, 2 * CHUNK, 3 * CHUNK, CPC]
                    w6q, w7q = [], []
                    for u in range(len(bounds) - 1):
                        c0, UW = bounds[u], bounds[u + 1] - bounds[u]
                        q6 = wqpool.tile([128, 6, UW], dt.float16,
                                         tag=f"w6u{len(bounds)}_{u}", name=f"w6u{u}",
                                         bufs=1)
                        q7 = wqpool.tile([128, UW], dt.float16,
                                         tag=f"w7u{len(bounds)}_{u}", name=f"w7u{u}",
                                         bufs=1)
                        nc.vector.tensor_add(
                            q6[:K], fan(xt[:K], 6 + c0, 6, -1, UW),
                            fan(xt[:K], 8 + c0, 6, 1, UW))
                        nc.gpsimd.tensor_add(
                            q7[:K], xt[:K, c0:c0 + UW],
                            xt[:K, 14 + c0:14 + c0 + UW])
                        w6q.append(q6)
                        w7q.append(q7)
                else:
                    w6 = wpool.tile([128, 6, CPC], dt.float16, tag="w6", name="w6")
                    w7 = wpool.tile([128, CPC], dt.float16, tag="w7", name="w7")
                    # wings computed in two column halves: PE's half-major
                    # matmul loop below then waits ~3.3us for the first half
                    # instead of ~6.5us for a whole-tile wing op
                    d7 = DVE_W7 // 2
                    for h0 in (0, HALF):
                        nc.vector.tensor_add(
                            w6[:K, :, h0:h0 + HALF],
                            fan(xt[:K], 6 + h0, 6, -1, HALF),
                            fan(xt[:K], 8 + h0, 6, 1, HALF))
                        # wing 7 column-split between DVE and Pool for balance
                        nc.vector.tensor_add(
                            w7[:K, h0:h0 + d7],
                            xt[:K, h0:h0 + d7], xt[:K, 14 + h0:14 + h0 + d7])
                        nc.gpsimd.tensor_add(
                            w7[:K, h0 + d7:h0 + HALF],
                            xt[:K, h0 + d7:h0 + HALF],
                            xt[:K, 14 + h0 + d7:14 + h0 + HALF])

                # previous tile's PSUM drain / base-mul / store, emitted here so
                # Pool's in-order queue runs w7(t) before mul(t-1)
                if pending is not None:
                    epilogue(tail=last)
                    pending = None

                ps_list = [ps_tile(c) for c in range(NCH)]
                if ends:
                    # chunk-major: each chunk's 8-band PSUM chain completes
                    # early; on the last tile drain+mul+store per chunk
                    # (pipelined tail instead of a serial whole-tile epilogue)
                    accp = {}
                    for c in range(NCH):
                        cs = slice(c * CHUNK, (c + 1) * CHUNK)
                        # wing unit covering this chunk's columns
                        u = max(i for i in range(len(bounds) - 1)
                                if bounds[i] <= c * CHUNK)
                        us = slice(c * CHUNK - bounds[u],
                                   (c + 1) * CHUNK - bounds[u])
                        for b in range(8):
                            if b == 0:
                                rhs = xt[:K, PAD + c * CHUNK:PAD + (c + 1) * CHUNK]
                            elif b < 7:
                                rhs = w6q[u][:K, b - 1, us]
                            else:
                                rhs = w7q[u][:K, us]
                            nc.tensor.matmul(
                                ps_list[c][:M, :],
                                bands_sb[:K, b * TILE_M:b * TILE_M + M],
                                rhs, start=(b == 0), stop=(b == 7))
                        if last:
                            # chunk-pair acc tiles: one store per pair halves
                            # the SP issue + HWDGE cost in the tail window
                            p = c // 2
                            if c % 2 == 0:
                                accp[p] = wqpool.tile(
                                    [128, 2 * CHUNK], dt.float16,
                                    tag=f"accp{p}", name=f"accp{p}", bufs=1)
                            accq = accp[p]
                            a = slice((c % 2) * CHUNK, (c % 2 + 1) * CHUNK)
                            if c < 2:
                                # c0/c1: ACT drain + Pool multiply (only
                                # wing-7 quarters load Pool in this tile)
                                nc.scalar.copy(accq[:M, a], ps_list[c][:M, :])
                                nc.gpsimd.tensor_mul(accq[:M, a], accq[:M, a],
                                                     bt[:M, cs])
                            else:
                                # c2/c3: fused DVE PSUM-read multiply -- the
                                # drain+sem round-trip would leave DVE idle
                                # right at the tail
                                nc.vector.tensor_mul(accq[:M, a], ps_list[c][:M, :],
                                                     bt[:M, cs])
                            if c == 1:
                                ps = slice(0, 2 * CHUNK)
                                nc.sync.dma_start(out_d[r0:r0 + M, ps], accq[:M, :])
                            elif c >= 2:
                                # separate stores for the last pair: the
                                # final post-matmul transfer covers only one
                                # chunk instead of two
                                nc.sync.dma_start(out_d[r0:r0 + M, cs], accq[:M, a])
                    if not last:
                        pending = (ps_list, bt, r0, M)
                else:
                    # half-major: each column half's 8-band pass depends only
                    # on that half's wing ops
                    for h in (0, 1):
                        for b in range(8):
                            if b == 0:
                                rhs = xt[:K, PAD:PAD + CPC]
                            elif b < 7:
                                rhs = w6[:K, b - 1]
                            else:
                                rhs = w7[:K, :]
                            lhsT = bands_sb[:K, b * TILE_M:b * TILE_M + M]
                            for c in (2 * h, 2 * h + 1):
                                nc.tensor.matmul(
                                    ps_list[c][:M, :],
                                    lhsT,
                                    rhs[:, c * CHUNK:(c + 1) * CHUNK],
                                    start=(b == 0),
                                    stop=(b == 7),
                                )
                    pending = (ps_list, bt, r0, M)
    return nc


def _split_sync_waits(nc):
    """Walrus codegen only supports one sync wait per instruction; hoist
    extra waits onto injected NoOps on the instruction's engine (identical
    semantics: the sequencer blocks at the NoOp first, then at the
    instruction).  DMA instructions are issued from their engine's
    sequencer stream, so the same hoisting applies to them.
    """
    import concourse.mybir as mybir

    n_nops = 0
    for fn in nc.m.functions:
        for bb in fn.blocks:
            new = []
            for inst in bb.instructions:
                si = inst.sync_info
                if si is not None and si.on_wait and len(si.on_wait) > 1:
                    waits = list(si.on_wait)
                    hoist, keep = waits[:-1], waits[-1:]
                    for w in hoist:
                        nop = mybir.InstNoOp(name=f"{inst.name}-w{n_nops}", ins=[], outs=[])
                        nop.engine = inst.engine
                        nop.sync_info = mybir.SyncInfo(on_wait=[w], on_update=[])
                        new.append(nop)
                        n_nops += 1
                    if hoist:
                        inst.sync_info = mybir.SyncInfo(
                            on_wait=keep, on_update=list(si.on_update))
                new.append(inst)
            bb.instructions = new
    return n_nops


def _get_nc():
    if "nc" not in _CACHE:
        nc = _build_nc()
        _split_sync_waits(nc)
        _CACHE["nc"] = nc
    return _CACHE["nc"]


def _run(x: np.ndarray, base_map: np.ndarray, trace: bool = False):
    from concourse.bass_utils import run_bass_kernel_spmd

    nc = _get_nc()
    xp = np.pad(np.asarray(x, dtype=np.float32), PAD, mode="edge").astype(F16)
    base16 = np.asarray(base_map, dtype=np.float32).astype(F16)
    bands = _bands_np()
    in_maps = []
    for c in range(N_CORES):
        gr, gc = divmod(c, GC)
        r0, c0 = gr * RPC, gc * CPC
        in_maps.append({
            "xb": np.ascontiguousarray(xp[r0:r0 + SHARD_R, c0:c0 + SHARD_C]),
            "base": np.ascontiguousarray(base16[r0:r0 + RPC, c0:c0 + CPC]),
            "bands": bands,
        })
    res = run_bass_kernel_spmd(nc, in_maps, list(range(N_CORES)), trace=trace)
    out = np.empty((H, W), dtype=np.float32)
    for c in range(N_CORES):
        gr, gc = divmod(c, GC)
        r0, c0 = gr * RPC, gc * CPC
        out[r0:r0 + RPC, c0:c0 + CPC] = res.results[c]["out"].astype(np.float32)
    return out[None, None], res


def kernel(x: np.ndarray, base_map: np.ndarray) -> np.ndarray:
    out, _ = _run(x, base_map, trace=False)
    return out


# revision 69
# speedup vs baseline: 1.0011x; 1.0011x over previous
"""Trainium2 Bass kernel for nn_Convolution_1176821039249.

Computes out = base_map * mean_k box_k(x) for k in {3,5,7,9,11,13,15} with
replicate padding, on 8 NeuronCores in a 4x2 spatial grid (1024x2048 per
core plus a 7-pixel halo on all sides, sliced host-side from the padded
full image, so no device-side halo exchange is needed).

Algorithm (per core):
  The total 2D kernel K(di,dj) = sum_k 1/(7k^2) 1[|di|<=k//2] 1[|dj|<=k//2]
  is decomposed over the horizontal "wing" basis
      T_0 = x(center),  T_m(j) = x(j-m) + x(j+m)   (m = 1..7)
  so that  out = sum_{b=0..7} P_b-vertical-band applied to T_b, where
      P_b(d) = sum_{k: k//2 >= max(b,|d|)} 1/(7k^2).
  Wings m=1..6 are fused DVE tensor_tensor adds (3D fan views with
  column-offset steps -1/+1); wing 7 is column-split between DVE and Pool
  to balance engine load.  The vertical pyramid bands are 8 PSUM-accumulated
  banded matmuls on the PE per 114-row tile (per-512-col-chunk PSUM tiles so
  chunk drains never block later matmuls); ACT drains PSUM to fp16, Pool
  multiplies by base_map, and the fp16 result is stored (host upconverts to
  fp32).

Scheduling: the pipeline is paced by the DVE wing chain (its per-tile work
matches the PE's), so mid tiles compute wings and consume them half-major
(two 1024-col halves) -- halving the producer->consumer latency granularity
removes all mid-stream PE stalls, which matters doubly because any PE idle
gap resets the p-state ramp (1 low + ~6 mid-rate matmuls per gap).  Tile 0
quarter-splits wings (matching the small leading slice of the first x load
so the DVE chain starts ASAP), tile 1 half-splits them, and the last tile
quarter-splits so its per-chunk epilogues pipeline.  A long warm-up matmul
chain keeps the PE busy (and the p-state warm) through the fill.  The tail
batches stores (pair-chunk acc tiles), runs early-chunk multiplies on Pool,
and fuses the last two chunks' drain+multiply into DVE PSUM-read ops, since
each tail store costs ~650ns SP issue + ~625ns HWDGE serialization and each
ACT drain adds a sem round-trip right where the kernel is latency-bound.
"""

import numpy as np

F16 = np.float16

H = W = 4096
PAD = 7
GR, GC = 4, 2               # core grid: 4 row-groups x 2 col-groups
RPC = H // GR               # 1024 output rows per core
CPC = W // GC               # 2048 output cols per core
SHARD_R = RPC + 2 * PAD     # 1038
SHARD_C = CPC + 2 * PAD     # 2062
TILE_M = 114                # output rows per tile (128 - 2*PAD)
N_TILES = 9                 # 8 * 114 + 112 = 1024
LAST_M = RPC - (N_TILES - 1) * TILE_M   # 112
CHUNK = 512                 # matmul N chunk (one PSUM bank of fp32)
NCH = CPC // CHUNK          # 4 chunks per tile
N_CORES = 8
KERNEL_SIZES = (3, 5, 7, 9, 11, 13, 15)
DVE_W7 = 600                # columns of wing-7 computed on DVE; rest on Pool

_CACHE = {}


def _bands_np() -> np.ndarray:
    """lhsT band matrices, [128, 8*TILE_M] fp16.

    Band b column i row p holds P_b(p - i - 7): the vertical pyramid profile
    applied to wing tensor T_b.
    """
    w = {k: 1.0 / (7.0 * k * k) for k in KERNEL_SIZES}
    P = np.zeros((8, 15), dtype=np.float64)
    for b in range(8):
        for d in range(-7, 8):
            P[b, d + 7] = sum(w[k] for k in KERNEL_SIZES if k // 2 >= max(b, abs(d)))
    M = np.zeros((128, 8 * TILE_M), dtype=np.float64)
    for b in range(8):
        for i in range(TILE_M):
            for p in range(i, i + 15):
                M[p, b * TILE_M + i] = P[b, p - i]
    return M.astype(F16)


def _build_nc():
    import concourse.bass as bass
    import concourse.mybir as mybir
    import concourse.tile as tile

    dt = mybir.dt

    nc = bass.Bass()
    xb_d = nc.declare_dram_parameter("xb", [SHARD_R, SHARD_C], dt.float16, isOutput=False)
    base_d = nc.declare_dram_parameter("base", [RPC, CPC], dt.float16, isOutput=False)
    bands_d = nc.declare_dram_parameter("bands", [128, 8 * TILE_M], dt.float16, isOutput=False)
    out_d = nc.declare_dram_parameter("out", [RPC, CPC], dt.float16, isOutput=True)

    with tile.TileContext(nc) as tc:
        with (
            tc.tile_pool(name="const", bufs=1) as constp,
            tc.tile_pool(name="xin", bufs=3) as xpool,
            tc.tile_pool(name="wings", bufs=3) as wpool,
            tc.tile_pool(name="wq", bufs=4) as wqpool,
            tc.tile_pool(name="io", bufs=3) as iopool,
            tc.tile_pool(name="psum", bufs=2, space="PSUM") as psump,
        ):
            def ps_tile(c):
                return psump.tile([128, CHUNK], dt.float32, tag=f"ps{c}",
                                  name=f"ps{c}")

            # PE p-state warm-up: a serialized chain of small matmuls (RAW on
            # the same PSUM region) keeps the PE continuously busy through the
            # DMA fill phase so real matmuls arrive in a warm >3us stretch at
            # full 2.4 GHz instead of cold-burst p-states
            warm = constp.tile([128, 128], dt.float16, name="warm")
            nc.gpsimd.memset(warm[:], 0.0)
            warm_ps = ps_tile(0)
            for _ in range(52):
                nc.tensor.matmul(warm_ps[:16, :128], warm[:, :16], warm[:, :128],
                                 start=True, stop=True)

            bands_sb = constp.tile([128, 8 * TILE_M], dt.float16, name="bands_sb")

            def fan(src, start, n, step, width):
                # [K, n, width] view: slice i covers columns
                # start + i*step .. +width (overlapping windows)
                v = src[:, start:start + width].unsqueeze(1)
                lst = v.ap
                lst[1] = (step, n)
                v.ap = lst
                return v

            pending = None  # (ps_list, bt, r0, M) awaiting drain+mul+store

            def epilogue(tail=False):
                ps_list, bt, r0, M = pending
                acc = iopool.tile([128, CPC], dt.float16, tag="acc", name="acc")
                if tail:
                    # final deferred tile: per-chunk drain -> DVE mul -> store
                    # (DVE idles once wings are done; Pool is busy with the
                    # last tile's w7 quarters)
                    for c in range(NCH):
                        cs = slice(c * CHUNK, (c + 1) * CHUNK)
                        nc.scalar.copy(acc[:M, cs], ps_list[c][:M, :])
                        nc.vector.tensor_mul(acc[:M, cs], acc[:M, cs], bt[:M, cs])
                        if c % 2 == 1:
                            # half-batched stores: halves the SP issue +
                            # HWDGE serialization in the tail window
                            hs = slice((c - 1) * CHUNK, (c + 1) * CHUNK)
                            nc.sync.dma_start(out_d[r0:r0 + M, hs], acc[:M, hs])
                    return
                for c in range(NCH):
                    cs = slice(c * CHUNK, (c + 1) * CHUNK)
                    nc.scalar.copy(acc[:M, cs], ps_list[c][:M, :])
                nc.gpsimd.tensor_mul(acc[:M, :], acc[:M, :], bt[:M, :])
                nc.sync.dma_start(out_d[r0:r0 + M, :], acc[:M, :])

            HALF = CPC // 2
            for t in range(N_TILES):
                M = TILE_M if t < N_TILES - 1 else LAST_M
                K = M + 2 * PAD
                r0 = t * TILE_M
                last = t == N_TILES - 1
                ends = t <= 1 or last

                xt = xpool.tile([128, SHARD_C], dt.float16, tag="xt", name="xt")
                if t == 0:
                    # split the first load (one small slice covering the
                    # first wing quarter, then the rest): the DVE chain that
                    # paces the whole pipeline starts on x quarter 0
                    FIRST = CHUNK + 2 * PAD
                    nc.sync.dma_start(xt[:K, :FIRST], xb_d[r0:r0 + K, :FIRST])
                    nc.sync.dma_start(xt[:K, FIRST:], xb_d[r0:r0 + K, FIRST:])
                else:
                    nc.sync.dma_start(xt[:K, :], xb_d[r0:r0 + K, :])
                bt = iopool.tile([128, CPC], dt.float16, tag="bt", name="bt")
                nc.sync.dma_start(bt[:M, :], base_d[r0:r0 + M, :])
                if t == 0:
                    nc.sync.dma_start(bands_sb[:], bands_d[:])

                # wings m=1..6 fused: fan slice i reads cols (base-i ..) and
                # (base'+i ..), i.e. x shifted by -(i+1)/+(i+1) around PAD=7
                if ends:
                    # split wings in per-unit tiles: chunk c's matmuls depend
                    # only on the unit covering its columns, shortening
                    # fill/tail.  Tile 0 leads with a chunk-wide unit (it
                    # matches the small leading x slice) then widens; tiles
                    # 1 and 8 use halves -- fewer op launches on the
                    # chain-critical DVE.
                    bounds = [0, HALF, CPC] if t == 1 \
                        else [0, CHUNK, 2 * CHUNK, 3 * CHUNK, CPC]
                    w6q, w7q = [], []
                    for u in range(len(bounds) - 1):
                        c0, UW = bounds[u], bounds[u + 1] - bounds[u]
                        q6 = wqpool.tile([128, 6, UW], dt.float16,
                                         tag=f"w6u{len(bounds)}_{u}", name=f"w6u{u}",
                                         bufs=1)
                        q7 = wqpool.tile([128, UW], dt.float16,
                                         tag=f"w7u{len(bounds)}_{u}", name=f"w7u{u}",
                                         bufs=1)
                        nc.vector.tensor_add(
                            q6[:K], fan(xt[:K], 6 + c0, 6, -1, UW),
                            fan(xt[:K], 8 + c0, 6, 1, UW))
                        nc.gpsimd.tensor_add(
                            q7[:K], xt[:K, c0:c0 + UW],
                            xt[:K, 14 + c0:14 + c0 + UW])
                        w6q.append(q6)
                        w7q.append(q7)
                else:
                    w6 = wpool.tile([128, 6, CPC], dt.float16, tag="w6", name="w6")
                    w7 = wpool.tile([128, CPC], dt.float16, tag="w7", name="w7")
                    # wings computed in two column halves: PE's half-major
                    # matmul loop below then waits ~3.3us for the first half
                    # instead of ~6.5us for a whole-tile wing op
                    d7 = DVE_W7 // 2
                    for h0 in (0, HALF):
                        nc.vector.tensor_add(
                            w6[:K, :, h0:h0 + HALF],
                            fan(xt[:K], 6 + h0, 6, -1, HALF),
                            fan(xt[:K], 8 + h0, 6, 1, HALF))
                        # wing 7 column-split between DVE and Pool for balance
                        nc.vector.tensor_add(
                            w7[:K, h0:h0 + d7],
                            xt[:K, h0:h0 + d7], xt[:K, 14 + h0:14 + h0 + d7])
                        nc.gpsimd.tensor_add(
                            w7[:K, h0 + d7:h0 + HALF],
                            xt[:K, h0 + d7:h0 + HALF],
                            xt[:K, 14 + h0 + d7:14 + h0 + HALF])

                # previous tile's PSUM drain / base-mul / store, emitted here so
                # Pool's in-order queue runs w7(t) before mul(t-1)
                if pending is not None:
                    epilogue(tail=last)
                    pending = None

                ps_list = [ps_tile(c) for c in range(NCH)]
                if ends:
                    # chunk-major: each chunk's 8-band PSUM chain completes
                    # early; on the last tile drain+mul+store per chunk
                    # (pipelined tail instead of a serial whole-tile epilogue)
                    accp = {}
                    for c in range(NCH):
                        cs = slice(c * CHUNK, (c + 1) * CHUNK)
                        # wing unit covering this chunk's columns
                        u = max(i for i in range(len(bounds) - 1)
                                if bounds[i] <= c * CHUNK)
                        us = slice(c * CHUNK - bounds[u],
                                   (c + 1) * CHUNK - bounds[u])
                        for b in range(8):
                            if b == 0:
                                rhs = xt[:K, PAD + c * CHUNK:PAD + (c + 1) * CHUNK]
                            elif b < 7:
                                rhs = w6q[u][:K, b - 1, us]
                            else:
                                rhs = w7q[u][:K, us]
                            nc.tensor.matmul(
                                ps_list[c][:M, :],
                                bands_sb[:K, b * TILE_M:b * TILE_M + M],
                                rhs, start=(b == 0), stop=(b == 7))
                        if last:
                            # chunk-pair acc tiles: one store per pair halves
                            # the SP issue + HWDGE cost in the tail window
                            p = c // 2
                            if c % 2 == 0:
                                accp[p] = wqpool.tile(
                                    [128, 2 * CHUNK], dt.float16,
                                    tag=f"accp{p}", name=f"accp{p}", bufs=1)
                            accq = accp[p]
                            a = slice((c % 2) * CHUNK, (c % 2 + 1) * CHUNK)
                            if c < 2:
                                # c0/c1: ACT drain + Pool multiply (only
                                # wing-7 quarters load Pool in this tile)
                                nc.scalar.copy(accq[:M, a], ps_list[c][:M, :])
                                nc.gpsimd.tensor_mul(accq[:M, a], accq[:M, a],
                                                     bt[:M, cs])
                            else:
                                # c2/c3: fused DVE PSUM-read multiply -- the
                                # drain+sem round-trip would leave DVE idle
                                # right at the tail
                                nc.vector.tensor_mul(accq[:M, a], ps_list[c][:M, :],
                                                     bt[:M, cs])
                            if c == 1:
                                ps = slice(0, 2 * CHUNK)
                                nc.sync.dma_start(out_d[r0:r0 + M, ps], accq[:M, :])
                            elif c >= 2:
                                # separate stores for the last pair: the
                                # final post-matmul transfer covers only one
                                # chunk instead of two
                                nc.sync.dma_start(out_d[r0:r0 + M, cs], accq[:M, a])
                    if not last:
                        pending = (ps_list, bt, r0, M)
                else:
                    # half-major: each column half's 8-band pass depends only
                    # on that half's wing ops
                    for h in (0, 1):
                        for b in range(8):
                            if b == 0:
                                rhs = xt[:K, PAD:PAD + CPC]
                            elif b < 7:
                                rhs = w6[:K, b - 1]
                            else:
                                rhs = w7[:K, :]
                            lhsT = bands_sb[:K, b * TILE_M:b * TILE_M + M]
                            for c in (2 * h, 2 * h + 1):
                                nc.tensor.matmul(
                                    ps_list[c][:M, :],
                                    lhsT,
                                    rhs[:, c * CHUNK:(c + 1) * CHUNK],
                                    start=(b == 0),
                                    stop=(b == 7),
                                )
                    pending = (ps_list, bt, r0, M)
    return nc


def _split_sync_waits(nc):
    """Walrus codegen only supports one sync wait per instruction; hoist
    extra waits onto injected NoOps on the instruction's engine (identical
    semantics: the sequencer blocks at the NoOp first, then at the
    instruction).  DMA instructions are issued from their engine's
    sequencer stream, so the same hoisting applies to them.
    """
    import concourse.mybir as mybir

    n_nops = 0
    for fn in nc.m.functions:
        for bb in fn.blocks:
            new = []
            for inst in bb.instructions:
                si = inst.sync_info
                if si is not None and si.on_wait and len(si.on_wait) > 1:
                    waits = list(si.on_wait)
                    hoist, keep = waits[:-1], waits[-1:]
                    for w in hoist:
                        nop = mybir.InstNoOp(name=f"{inst.name}-w{n_nops}", ins=[], outs=[])
                        nop.engine = inst.engine
                        nop.sync_info = mybir.SyncInfo(on_wait=[w], on_update=[])
                        new.append(nop)
                        n_nops += 1
                    if hoist:
                        inst.sync_info = mybir.SyncInfo(
                            on_wait=keep, on_update=list(si.on_update))
                new.append(inst)
            bb.instructions = new
    return n_nops


def _get_nc():
    if "nc" not in _CACHE:
        nc = _build_nc()
        _split_sync_waits(nc)
        _CACHE["nc"] = nc
    return _CACHE["nc"]


def _run(x: np.ndarray, base_map: np.ndarray, trace: bool = False):
    from concourse.bass_utils import run_bass_kernel_spmd

    nc = _get_nc()
    xp = np.pad(np.asarray(x, dtype=np.float32), PAD, mode="edge").astype(F16)
    base16 = np.asarray(base_map, dtype=np.float32).astype(F16)
    bands = _bands_np()
    in_maps = []
    for c in range(N_CORES):
        gr, gc = divmod(c, GC)
        r0, c0 = gr * RPC, gc * CPC
        in_maps.append({
            "xb": np.ascontiguousarray(xp[r0:r0 + SHARD_R, c0:c0 + SHARD_C]),
            "base": np.ascontiguousarray(base16[r0:r0 + RPC, c0:c0 + CPC]),
            "bands": bands,
        })
    res = run_bass_kernel_spmd(nc, in_maps, list(range(N_CORES)), trace=trace)
    out = np.empty((H, W), dtype=np.float32)
    for c in range(N_CORES):
        gr, gc = divmod(c, GC)
        r0, c0 = gr * RPC, gc * CPC
        out[r0:r0 + RPC, c0:c0 + CPC] = res.results[c]["out"].astype(np.float32)
    return out[None, None], res


def kernel(x: np.ndarray, base_map: np.ndarray) -> np.ndarray:
    out, _ = _run(x, base_map, trace=False)
    return out
